# revision 1
# baseline (speedup 1.0000x reference)
"""GCN encoder (2x GCNConv + ReLU + AdaptiveAvgPool) on 8 Trainium2 NeuronCores.

Math (matches reference):
    deg[i]  = #edges with dst==i (+1 self loop);  dinv = deg^-1/2
    h       = relu( A_norm @ (x @ W1) + b1 ),  A_norm = D^-1/2 (A+I) D^-1/2
    out2    = A_norm @ (h @ W2) + b2
    pooled[g] = mean over nodes n in group g (1600 nodes) of out2[n]

Key algebraic restructurings (exact, fp-reassociation only):
  * W1 commutes with aggregation: A_norm @ (x@W1) = (A_norm @ x) @ W1,
    so the per-edge payload is one 16-float x row, not 64.
  * A_norm factorizes: agg[d] = dinv[d] * sum_{e->d} xd[src_e],
    xd = dinv[:,None]*x.  No per-edge weights on device.
  * The pooled output only needs z[g] = sum_n C[n,g] * (dinv[n] h[n])
    with C[n,g] = sum_{e: src=n, dst in g} dinv[dst_e] (host-built
    graph metadata).  pooled = (z @ W2)/1600 + b2.

Gather strategy (HW-measured constraints, see memory notes):
  indirect_dma_start is limited to 128 rows/instr (~1.2us each) and
  dma_gather to 8192 int16 indices of 256B elements per instr, one SDMA
  engine (~22 GB/s) per SWDGE queue, 4 queues scale ~linearly (~81 GB/s).
  So: quad-packed table xdq[N/4+1, 64] f32 (4 consecutive xd rows = 256B,
  quad index = src>>2 < 32768 fits int16, last row zeros for padding),
  bulk dma_gather over 4 queues, then a host-baked per-slot mask
  (dinv[dst] x onehot(src&3)) selects the right 16-float row during a
  fused DVE multiply + free-axis segment reduce.
Device work per core: the 256B-element gather (~31 MB random reads),
mask-mult + segment reduce, transpose, @W1+b1, relu*dinv, z psum-accum,
(z@W2+200*b2)/1600.  Host combines: output = sum of 8 partials.
"""

import numpy as np

N = 51200
E = 819200
F = 16          # input feats
H1 = 64         # hidden
H2 = 128        # output feats
G = 32          # pool groups
GS = N // G     # 1600 nodes per group
NCORES = 8
NPC = N // NCORES       # nodes per core: 6400
NBLK = NPC // 128       # 50 blocks of 128 nodes
PAD_IDX = 10_000_000    # host-side empty-slot marker
# quad-table zeros row index is N//4 (computed at call time)
GCOLS = 64              # slot columns per dma_gather (64*128 = 8192 idxs)
CHUNK_G = 2             # gathers per chunk
CHUNK_COLS = GCOLS * CHUNK_G


def _prep(x, edge_index, W1, b1, W2, b2):
    """Host-side graph preprocessing: degrees, norms, C matrix, per-core
    quad-index + mask slot tables.  Returns (static_cfg, per_core_inmaps)."""
    src = edge_index[0].astype(np.int64)
    dst = edge_index[1].astype(np.int64)

    deg_e = np.bincount(dst, minlength=N)           # edge in-degree
    deg = deg_e + 1                                 # + self loop
    dinv = (1.0 / np.sqrt(deg.astype(np.float64))).astype(np.float32)

    xd = (x.astype(np.float32) * dinv[:, None]).astype(np.float32)
    xdq = np.zeros((N // 4 + 1, 4 * F), np.float32)
    xdq[:N // 4] = xd.reshape(N // 4, 4 * F)

    # C[n, g] = sum_{e: src=n, dst//GS=g} dinv[dst]  (+ self loop term)
    g_e = dst // GS
    C = np.bincount(src * G + g_e, weights=dinv[dst].astype(np.float64),
                    minlength=N * G).astype(np.float32).reshape(N, G)
    C[np.arange(N), np.arange(N) // GS] += dinv

    # dst-sorted source table, padded per node
    order_e = np.argsort(dst, kind="stable")
    srcs_sorted = src[order_e].astype(np.int32)
    maxdeg_e = int(deg_e.max())
    Tw = ((maxdeg_e + 1 + 3) // 4) * 4
    T = np.full((N, Tw), PAD_IDX, np.int32)
    T[:, 0] = np.arange(N, dtype=np.int32)          # self loop slot
    mask = np.arange(Tw - 1)[None, :] < deg_e[:, None]
    T[:, 1:][mask] = srcs_sorted

    # degree-sorted, strided node->core assignment (per-block caps then
    # match across cores -> one SPMD program)
    order_n = np.argsort(deg, kind="stable")
    cores_nodes = [order_n[c::NCORES] for c in range(NCORES)]

    caps = []
    for B in range(NBLK):
        m = 0
        for c in range(NCORES):
            nodes = cores_nodes[c][B * 128:(B + 1) * 128]
            m = max(m, int(deg[nodes].max()))
        caps.append(m)

    # chunk assignment: whole blocks per chunk, <= CHUNK_COLS slot columns
    chunks = []          # (b0, b1, col0)
    col0, b0, acc = 0, 0, 0
    for B in range(NBLK):
        if acc + caps[B] > CHUNK_COLS:
            chunks.append((b0, B, col0))
            col0 += CHUNK_COLS
            b0, acc = B, 0
        acc += caps[B]
    chunks.append((b0, NBLK, col0))
    SP = col0 + CHUNK_COLS          # padded total slot columns
    NG = SP // GCOLS                # total gathers
    boff = {}
    for (bb0, bb1, c0) in chunks:
        c = c0
        for B in range(bb0, bb1):
            boff[B] = c
            c += caps[B]

    w1 = np.ascontiguousarray(W1.astype(np.float32))
    b1r = np.ascontiguousarray(b1.astype(np.float32).reshape(1, H1))
    w2 = np.ascontiguousarray(W2.astype(np.float32))
    b2r = np.ascontiguousarray(b2.astype(np.float32).reshape(1, H2))

    per_core = []
    for c in range(NCORES):
        qidx_cols = np.full((128, SP), N // 4, np.int32)
        mask_all = np.zeros((128, SP, 4), np.float32)
        dinv_pos = np.zeros((128, NBLK), np.float32)
        c_all = np.zeros((128, NBLK * G), np.float32)
        for B in range(NBLK):
            nodes = cores_nodes[c][B * 128:(B + 1) * 128]
            cap = caps[B]
            st = T[nodes, :cap]
            pad = st == PAD_IDX
            o = boff[B]
            qidx_cols[:, o:o + cap] = np.where(pad, N // 4, st >> 2)
            sub = np.where(pad, 0, st & 3)
            m = (np.arange(4)[None, None, :] == sub[:, :, None]).astype(
                np.float32)
            m[pad] = 0.0
            m *= dinv[nodes][:, None, None]
            mask_all[:, o:o + cap, :] = m
            dinv_pos[:, B] = dinv[nodes]
            c_all[:, B * G:(B + 1) * G] = C[nodes]
        # per-gather wrap-16 int16 index streams, replicated to 128 parts.
        # Chunk-tail columns (after the last block) become trailing -1s so
        # the firmware skips their descriptors (num_idxs_reg = valid count).
        gidx = np.zeros((128, NG * (GCOLS * 128 // 16)), np.int16)
        W16 = GCOLS * 128 // 16      # 512 idx columns per gather
        vend = {}
        for (bb0, bb1, cc0) in chunks:
            vend[cc0 // CHUNK_COLS] = boff[bb1 - 1] + caps[bb1 - 1]
        gvalid = []
        for g in range(NG):
            nv = min(GCOLS, max(0, vend[g * GCOLS // CHUNK_COLS] - g * GCOLS))
            gvalid.append(nv * 128)
            pos = qidx_cols[:, g * GCOLS:(g + 1) * GCOLS]   # (128, 64)
            pv = pos.T.ravel().copy()                       # position order
            pv[nv * 128:] = -1
            arr = pv.reshape(W16, 16).T.astype(np.int16)    # (16, 512)
            gidx[:, g * W16:(g + 1) * W16] = np.tile(arr, (8, 1))
        per_core.append(dict(
            xdq=xdq, gidx=gidx,
            mask_all=mask_all.reshape(128, SP * 4),
            dinv_pos=dinv_pos, c_all=c_all,
            w1=w1, b1r=b1r, w2=w2, b2r=b2r,
        ))

    cfg = (tuple(caps), tuple(chunks), SP, tuple(gvalid))
    return cfg, per_core


def _build(cfg, nrep=1):
    # nrep > 1 duplicates the pipeline body (timing only; output invalid).
    import concourse.bass as bass
    import concourse.bacc as bacc
    import concourse.tile as tile
    from concourse import mybir
    from concourse.masks import make_identity

    caps, chunks, SP, gvalid = cfg
    NG = SP // GCOLS
    W16 = GCOLS * 128 // 16
    boff = {}
    for (bb0, bb1, c0) in chunks:
        c = c0
        for B in range(bb0, bb1):
            boff[B] = c
            c += caps[B]

    f32 = mybir.dt.float32
    i16 = mybir.dt.int16

    nc = bacc.Bacc("TRN2", target_bir_lowering=False, debug=False,
                   num_devices=NCORES, num_swdge_queues=4)

    xdq_t = nc.dram_tensor("xdq", [N // 4 + 1, 4 * F], f32,
                           kind="ExternalInput")
    gidx_t = nc.dram_tensor("gidx", [128, NG * W16], i16,
                            kind="ExternalInput")
    mask_t = nc.dram_tensor("mask_all", [128, SP * 4], f32,
                            kind="ExternalInput")
    dinv_t = nc.dram_tensor("dinv_pos", [128, NBLK], f32,
                            kind="ExternalInput")
    c_t = nc.dram_tensor("c_all", [128, NBLK * G], f32, kind="ExternalInput")
    w1_t = nc.dram_tensor("w1", [F, H1], f32, kind="ExternalInput")
    b1_t = nc.dram_tensor("b1r", [1, H1], f32, kind="ExternalInput")
    w2_t = nc.dram_tensor("w2", [H1, H2], f32, kind="ExternalInput")
    b2_t = nc.dram_tensor("b2r", [1, H2], f32, kind="ExternalInput")
    out_t = nc.dram_tensor("p_out", [G, H2], f32, kind="ExternalOutput")

    AF = mybir.ActivationFunctionType
    AX = mybir.AxisListType
    OP = mybir.AluOpType

    with tile.TileContext(nc) as tc:
        with tc.tile_pool(name="const", bufs=1) as constp, \
             tc.tile_pool(name="stream", bufs=2) as streamp, \
             tc.tile_pool(name="work", bufs=3) as workp, \
             tc.tile_pool(name="psum", bufs=2, space="PSUM") as psump, \
             tc.tile_pool(name="psumacc", bufs=1, space="PSUM") as psumaccp:

            ident = constp.tile([128, 128], f32)
            make_identity(nc, ident[:])
            ones_row = constp.tile([1, 128], f32)
            nc.vector.memset(ones_row[:], 1.0)
            ones_b2 = constp.tile([1, G], f32)
            nc.vector.memset(ones_b2[:], float(GS) / NCORES)   # 200.0

            w1 = constp.tile([F, H1], f32)
            nc.sync.dma_start(out=w1[:], in_=w1_t[:, :])
            b1s = constp.tile([1, H1], f32)
            nc.sync.dma_start(out=b1s[:], in_=b1_t[:, :])
            w2 = constp.tile([H1, H2], f32)
            nc.sync.dma_start(out=w2[:], in_=w2_t[:, :])
            b2s = constp.tile([1, H2], f32)
            nc.sync.dma_start(out=b2s[:], in_=b2_t[:, :])
            dinv = constp.tile([128, NBLK], f32)
            nc.sync.dma_start(out=dinv[:], in_=dinv_t[:, :])
            call = constp.tile([128, NBLK * G], f32)
            nc.sync.dma_start(out=call[:], in_=c_t[:, :])

            psum_zT = psumaccp.tile([H1, G], f32)

            gq = 0
            rep_chunks = [c for _ in range(nrep) for c in chunks]
            for ci, (b0, b1_, c0) in enumerate(rep_chunks):
                gci = (c0 // GCOLS) % NG
                idxg = streamp.tile([128, CHUNK_G * W16], i16, tag="idx")
                nc.sync.dma_start(
                    out=idxg[:],
                    in_=gidx_t[:, gci * W16:(gci + CHUNK_G) * W16])
                maskt = streamp.tile([128, CHUNK_COLS * 4], f32, tag="mask")
                nc.sync.dma_start(out=maskt[:],
                                  in_=mask_t[:, c0 * 4:(c0 + CHUNK_COLS) * 4])
                gchunk = streamp.tile([128, CHUNK_COLS * 4 * F], f32,
                                      tag="gbuf", bufs=4)
                for gi in range(CHUNK_G):
                    if gvalid[gci + gi] == 0:
                        continue
                    nc.gpsimd.dma_gather(
                        out_ap=gchunk[:, gi * GCOLS * 4 * F:
                                      (gi + 1) * GCOLS * 4 * F].rearrange(
                            "p (s f) -> p s f", f=4 * F),
                        in_ap=xdq_t[:, :],
                        idxs_ap=idxg[:, gi * W16:(gi + 1) * W16],
                        num_idxs=GCOLS * 128,
                        num_idxs_reg=gvalid[gci + gi],
                        elem_size=4 * F, single_packet=False,
                        queue_num=gq % 4)
                    gq += 1
                for B in range(b0, b1_):
                    o = boff[B] - c0
                    cap = caps[B]
                    gv = gchunk[:, o * 4 * F:(o + cap) * 4 * F].rearrange(
                        "p (c j f) -> p c j f", j=4, f=F)
                    mv = maskt[:, o * 4:(o + cap) * 4].rearrange(
                        "p (c j) -> p c j", j=4)
                    mvb = bass.AP(mv.tensor, mv.offset,
                                  list(mv.ap) + [[0, F]])
                    # in-place: select the right sub-row and fold dinv[dst]
                    nc.vector.tensor_tensor(out=gv, in0=gv, in1=mvb,
                                            op=OP.mult)
                    agg = workp.tile([128, F], f32, tag="agg")
                    nc.vector.tensor_reduce(
                        agg[:],
                        gchunk[:, o * 4 * F:(o + cap) * 4 * F].rearrange(
                            "p (s f) -> p f s", f=F),
                        axis=AX.X, op=OP.add)
                    pt = psump.tile([F, 128], f32, tag="pt")
                    nc.tensor.transpose(out=pt[:], in_=agg[:],
                                        identity=ident[:])
                    aggT = workp.tile([F, 128], f32, tag="aggT")
                    nc.scalar.copy(aggT[:], pt[:])
                    ph = psump.tile([128, H1], f32, tag="ph")
                    nc.tensor.matmul(out=ph[:], lhsT=aggT[:], rhs=w1[:],
                                     start=True, stop=False)
                    nc.tensor.matmul(out=ph[:], lhsT=ones_row[:], rhs=b1s[:],
                                     start=False, stop=True)
                    hd = workp.tile([128, H1], f32, tag="hd")
                    # relu(y * dinv) == dinv * relu(y) since dinv > 0
                    nc.scalar.activation(hd[:], ph[:], AF.Relu,
                                         scale=dinv[:, B:B + 1])
                    nc.tensor.matmul(out=psum_zT[:], lhsT=hd[:],
                                     rhs=call[:, B * G:(B + 1) * G],
                                     start=(ci == 0 and B == b0),
                                     stop=(ci == len(rep_chunks) - 1
                                           and B == b1_ - 1),
                                     skip_group_check=True)

            zT = constp.tile([H1, G], f32)
            nc.scalar.copy(zT[:], psum_zT[:])
            pP = psump.tile([G, H2], f32, tag="pP")
            nc.tensor.matmul(out=pP[:], lhsT=zT[:], rhs=w2[:],
                             start=True, stop=False)
            nc.tensor.matmul(out=pP[:], lhsT=ones_b2[:], rhs=b2s[:],
                             start=False, stop=True)
            pout = constp.tile([G, H2], f32)
            nc.scalar.activation(pout[:], pP[:], AF.Copy, scale=1.0 / GS)
            nc.sync.dma_start(out=out_t[:, :], in_=pout[:])

    nc.compile()
    return nc


_CACHE = {}


def kernel(**inputs):
    x = np.asarray(inputs["x"], dtype=np.float32)
    edge_index = np.asarray(inputs["edge_index"])
    W1 = np.asarray(inputs["W1"], dtype=np.float32)
    b1 = np.asarray(inputs["b1"], dtype=np.float32)
    W2 = np.asarray(inputs["W2"], dtype=np.float32)
    b2 = np.asarray(inputs["b2"], dtype=np.float32)
    assert x.shape == (N, F) and edge_index.shape == (2, E)

    cfg, per_core = _prep(x, edge_index, W1, b1, W2, b2)

    from concourse.bass_utils import run_bass_kernel_spmd

    if cfg not in _CACHE:
        _CACHE[cfg] = _build(cfg)
    nc = _CACHE[cfg]

    res = run_bass_kernel_spmd(nc, per_core, list(range(NCORES)))
    out = np.zeros((G, H2), np.float64)
    for r in res.results:
        out += r["p_out"].astype(np.float64)
    return out.astype(np.float32).reshape(1, G, H2)


if __name__ == "__main__":
    rng = np.random.default_rng(0)
    ins = dict(
        x=rng.standard_normal((N, F), dtype=np.float32),
        edge_index=rng.integers(0, N, (2, E)).astype(np.int32),
        W1=rng.standard_normal((F, H1), dtype=np.float32) * 0.25,
        b1=np.zeros(H1, np.float32),
        W2=rng.standard_normal((F if False else H1, H2),
                               dtype=np.float32) * 0.125,
        b2=np.zeros(H2, np.float32),
    )
    out = kernel(**ins)
    print(out.shape, out.dtype, float(np.abs(out).mean()))



# revision 2
# speedup vs baseline: 270.2132x; 270.2132x over previous
"""GCN encoder (2x GCNConv + ReLU + AdaptiveAvgPool) on 8 Trainium2 cores, v2.

Same math/structure as the v1 kernel (see kernel.py docstring): per-edge
payload is one dinv-scaled x row; the second GCN layer + pooling collapse
into a host-built C matrix (pure graph structure), so device work is
  agg = gather+mask+reduce;  h = relu((agg @ W1 + b1) * dinv)
  zT += h^T-ish psum accum via C;  out = (z @ W2)/1600 + b2.

v2 changes vs v1 (HW-measured):
  * The SWDGE gather is HBM random-TRANSACTION limited (~400M/s/core);
    element size barely matters. So rows are PAIR-packed (2 nodes = 128B)
    at 256B stride instead of QUAD-packed 256B: ~15% faster gather, int16
    idx = src>>1 < 25600, and the DVE select/reduce work halves.
    (bass.py's 256B-multiple elem_size assert is a transpose-path
    restriction; the Q7 desc-gen handles any elem_size - emitted directly.)
  * gidx and masks are loaded in full before the gather stream starts, so
    all 16 gathers fire back-to-back with no per-chunk DMA dependencies.
"""

import numpy as np

N = 51200
E = 819200
F = 16          # input feats
H1 = 64         # hidden
H2 = 128        # output feats
G = 32          # pool groups
GS = N // G     # 1600 nodes per group
NCORES = 8
NPC = N // NCORES       # nodes per core: 6400
NBLK = NPC // 128       # 50 blocks of 128 nodes
PAD_IDX = 10_000_000    # host-side empty-slot marker
GCOLS = 64              # slot columns per dma_gather (64*128 = 8192 idxs)
CHUNK_G = 2             # gathers per chunk
CHUNK_COLS = GCOLS * CHUNK_G
PROW = N // 2           # pair-table rows (zeros row index)


def _prep(x, edge_index, W1, b1, W2, b2):
    """Host-side graph preprocessing (structure only, plus the elementwise
    dinv scaling of x): degrees, norms, C matrix, per-core pair-index +
    2-way select-mask tables.  Returns (static_cfg, per_core_inmaps)."""
    src = edge_index[0].astype(np.int64)
    dst = edge_index[1].astype(np.int64)

    deg_e = np.bincount(dst, minlength=N)           # edge in-degree
    deg = deg_e + 1                                 # + self loop
    dinv = (1.0 / np.sqrt(deg.astype(np.float64))).astype(np.float32)

    xd = (x.astype(np.float32) * dinv[:, None]).astype(np.float32)
    # pair-packed rows at 256B pitch: row r = nodes (2r, 2r+1), 32 f32 used
    xdp = np.zeros((PROW + 1, 64), np.float32)
    xdp[:PROW, :2 * F] = xd.reshape(PROW, 2 * F)

    # C[n, g] = sum_{e: src=n, dst//GS=g} dinv[dst]  (+ self loop term)
    g_e = dst // GS
    C = np.bincount(src * G + g_e, weights=dinv[dst].astype(np.float64),
                    minlength=N * G).astype(np.float32).reshape(N, G)
    C[np.arange(N), np.arange(N) // GS] += dinv

    # dst-sorted source table, padded per node
    order_e = np.argsort(dst, kind="stable")
    srcs_sorted = src[order_e].astype(np.int32)
    maxdeg_e = int(deg_e.max())
    Tw = ((maxdeg_e + 1 + 3) // 4) * 4
    T = np.full((N, Tw), PAD_IDX, np.int32)
    T[:, 0] = np.arange(N, dtype=np.int32)          # self loop slot
    mask = np.arange(Tw - 1)[None, :] < deg_e[:, None]
    T[:, 1:][mask] = srcs_sorted

    # degree-sorted, strided node->core assignment (per-block caps then
    # match across cores -> one SPMD program)
    order_n = np.argsort(deg, kind="stable")
    cores_nodes = [order_n[c::NCORES] for c in range(NCORES)]

    caps = []
    for B in range(NBLK):
        m = 0
        for c in range(NCORES):
            nodes = cores_nodes[c][B * 128:(B + 1) * 128]
            m = max(m, int(deg[nodes].max()))
        caps.append(m)

    # chunk assignment: whole blocks per chunk, <= CHUNK_COLS slot columns
    chunks = []          # (b0, b1, col0)
    col0, b0, acc = 0, 0, 0
    for B in range(NBLK):
        if acc + caps[B] > CHUNK_COLS:
            chunks.append((b0, B, col0))
            col0 += CHUNK_COLS
            b0, acc = B, 0
        acc += caps[B]
    chunks.append((b0, NBLK, col0))
    SP = col0 + CHUNK_COLS          # padded total slot columns
    NG = SP // GCOLS                # total gathers
    boff = {}
    for (bb0, bb1, c0) in chunks:
        c = c0
        for B in range(bb0, bb1):
            boff[B] = c
            c += caps[B]

    w1 = np.ascontiguousarray(W1.astype(np.float32))
    b1r = np.ascontiguousarray(b1.astype(np.float32).reshape(1, H1))
    w2 = np.ascontiguousarray(W2.astype(np.float32))
    b2r = np.ascontiguousarray(b2.astype(np.float32).reshape(1, H2))

    per_core = []
    for c in range(NCORES):
        qidx_cols = np.full((128, SP), PROW, np.int32)
        mask_all = np.zeros((128, SP, 2), np.float32)
        dinv_pos = np.zeros((128, NBLK), np.float32)
        c_all = np.zeros((128, NBLK * G), np.float32)
        for B in range(NBLK):
            nodes = cores_nodes[c][B * 128:(B + 1) * 128]
            cap = caps[B]
            st = T[nodes, :cap]
            pad = st == PAD_IDX
            o = boff[B]
            qidx_cols[:, o:o + cap] = np.where(pad, PROW, st >> 1)
            sub = np.where(pad, 0, st & 1)
            m = (np.arange(2)[None, None, :] == sub[:, :, None]).astype(
                np.float32)
            m[pad] = 0.0
            m *= dinv[nodes][:, None, None]
            mask_all[:, o:o + cap, :] = m
            dinv_pos[:, B] = dinv[nodes]
            c_all[:, B * G:(B + 1) * G] = C[nodes]
        # per-gather wrap-16 int16 index streams, replicated to 128 parts.
        # Chunk-tail columns (after the last block) become trailing -1s so
        # the firmware skips their descriptors (num_idxs_reg = valid count).
        gidx = np.zeros((128, NG * (GCOLS * 128 // 16)), np.int16)
        W16 = GCOLS * 128 // 16      # 512 idx columns per gather
        vend = {}
        for (bb0, bb1, cc0) in chunks:
            vend[cc0 // CHUNK_COLS] = boff[bb1 - 1] + caps[bb1 - 1]
        gvalid = []
        for g in range(NG):
            nv = min(GCOLS, max(0, vend[g * GCOLS // CHUNK_COLS] - g * GCOLS))
            gvalid.append(nv * 128)
            pos = qidx_cols[:, g * GCOLS:(g + 1) * GCOLS]   # (128, 64)
            pv = pos.T.ravel().copy()                       # position order
            pv[nv * 128:] = -1
            arr = pv.reshape(W16, 16).T.astype(np.int16)    # (16, 512)
            gidx[:, g * W16:(g + 1) * W16] = np.tile(arr, (8, 1))
        per_core.append(dict(
            xdp=xdp, gidx=gidx,
            mask_all=mask_all.reshape(128, SP * 2),
            dinv_pos=dinv_pos, c_all=c_all,
            w1=w1, b1r=b1r, w2=w2, b2r=b2r,
        ))

    cfg = (tuple(caps), tuple(chunks), SP, tuple(gvalid))
    return cfg, per_core


def _emit_gather128(gp, out_ap, in_ap, idxs_ap, num_idxs, num_idxs_reg,
                    elem_size, elem_step, single_packet, queue_num):
    """dma_gather with elem_size_bytes < 256 (any size; 256B-multiple
    stride).  bass.py's % 256 assert is a transpose-path restriction -
    the Q7 non-transpose desc-gen packs min(elem, 256B) packets."""
    from concourse import mybir
    from concourse.bass import exact_div

    gp._assert_queue_num(queue_num)
    stride_bytes = elem_step * mybir.dt.size(in_ap.dtype)
    _in_ap = gp.lower_ap_dma(in_ap, for_custom_bir_dma=True)
    _idxs_ap = gp.lower_ap(idxs_ap)
    _out_ap = gp.lower_ap(out_ap)
    return gp.add_instruction(
        mybir.InstDMAGatherAnt(
            name=gp.bass.get_next_instruction_name(),
            ins=[*_in_ap, _idxs_ap,
                 gp.lower_val_access(gp.to_reg(num_idxs_reg))],
            outs=[_out_ap],
            transpose=False,
            num_idxs=num_idxs,
            elem_size=elem_size,
            stride_bytes_256=exact_div(stride_bytes, 256),
            gen_mode=0,
            single_packet=single_packet,
            queue_num=queue_num,
            sbuf_tokens_per_rank=0,
            sbuf_free_dim_per_rank=0,
            sbuf_free_dim_pad_per_rank=0,
            sbuf_byte_offset=0,
        ))


def _build(cfg, nrep=1):
    # nrep > 1 duplicates the pipeline body (timing only; output invalid).
    import concourse.bass as bass
    import concourse.bacc as bacc
    import concourse.tile as tile
    from concourse import mybir
    from concourse.masks import make_identity

    caps, chunks, SP, gvalid = cfg
    NG = SP // GCOLS
    W16 = GCOLS * 128 // 16
    boff = {}
    for (bb0, bb1, c0) in chunks:
        c = c0
        for B in range(bb0, bb1):
            boff[B] = c
            c += caps[B]

    f32 = mybir.dt.float32
    i16 = mybir.dt.int16

    nc = bacc.Bacc("TRN2", target_bir_lowering=False, debug=False,
                   num_devices=NCORES, num_swdge_queues=4)

    xdp_t = nc.dram_tensor("xdp", [PROW + 1, 64], f32, kind="ExternalInput")
    gidx_t = nc.dram_tensor("gidx", [128, NG * W16], i16,
                            kind="ExternalInput")
    mask_t = nc.dram_tensor("mask_all", [128, SP * 2], f32,
                            kind="ExternalInput")
    dinv_t = nc.dram_tensor("dinv_pos", [128, NBLK], f32,
                            kind="ExternalInput")
    c_t = nc.dram_tensor("c_all", [128, NBLK * G], f32, kind="ExternalInput")
    w1_t = nc.dram_tensor("w1", [F, H1], f32, kind="ExternalInput")
    b1_t = nc.dram_tensor("b1r", [1, H1], f32, kind="ExternalInput")
    w2_t = nc.dram_tensor("w2", [H1, H2], f32, kind="ExternalInput")
    b2_t = nc.dram_tensor("b2r", [1, H2], f32, kind="ExternalInput")
    out_t = nc.dram_tensor("p_out", [G, H2], f32, kind="ExternalOutput")

    AF = mybir.ActivationFunctionType
    AX = mybir.AxisListType
    OP = mybir.AluOpType

    with tile.TileContext(nc) as tc:
        with tc.tile_pool(name="const", bufs=1) as constp, \
             tc.tile_pool(name="work", bufs=3) as workp, \
             tc.tile_pool(name="gbuf", bufs=1) as gbufp, \
             tc.tile_pool(name="psum", bufs=2, space="PSUM") as psump, \
             tc.tile_pool(name="psumacc", bufs=1, space="PSUM") as psumaccp:

            ident = constp.tile([128, 128], f32)
            make_identity(nc, ident[:])
            ones_row = constp.tile([1, 128], f32)
            nc.vector.memset(ones_row[:], 1.0)
            ones_b2 = constp.tile([1, G], f32)
            nc.vector.memset(ones_b2[:], float(GS) / NCORES)   # 200.0

            # whole idx + mask images up front: the gather stream then has
            # no intra-rep DMA dependencies at all.
            gidx = constp.tile([128, NG * W16], i16)
            nc.sync.dma_start(out=gidx[:], in_=gidx_t[:, :])
            maskt = constp.tile([128, SP * 2], f32)
            nc.sync.dma_start(out=maskt[:], in_=mask_t[:, :])

            w1 = constp.tile([F, H1], f32)
            nc.sync.dma_start(out=w1[:], in_=w1_t[:, :])
            b1s = constp.tile([1, H1], f32)
            nc.sync.dma_start(out=b1s[:], in_=b1_t[:, :])
            w2 = constp.tile([H1, H2], f32)
            nc.sync.dma_start(out=w2[:], in_=w2_t[:, :])
            b2s = constp.tile([1, H2], f32)
            nc.sync.dma_start(out=b2s[:], in_=b2_t[:, :])
            dinv = constp.tile([128, NBLK], f32)
            nc.sync.dma_start(out=dinv[:], in_=dinv_t[:, :])
            call = constp.tile([128, NBLK * G], f32)
            nc.sync.dma_start(out=call[:], in_=c_t[:, :])

            psum_zT = psumaccp.tile([H1, G], f32)

            gq = 0
            rep_chunks = [c for _ in range(nrep) for c in chunks]
            for ci, (b0, b1_, c0) in enumerate(rep_chunks):
                gci = (c0 // GCOLS) % NG
                gchunk = gbufp.tile([128, CHUNK_COLS * 2 * F], f32,
                                    tag="gbuf", bufs=4)
                for gi in range(CHUNK_G):
                    if gvalid[gci + gi] == 0:
                        continue
                    _emit_gather128(
                        nc.gpsimd,
                        out_ap=gchunk[:, gi * GCOLS * 2 * F:
                                      (gi + 1) * GCOLS * 2 * F].rearrange(
                            "p (s f) -> p s f", f=2 * F),
                        in_ap=xdp_t[:, :2 * F],
                        idxs_ap=gidx[:, (gci + gi) * W16:
                                     (gci + gi + 1) * W16],
                        num_idxs=GCOLS * 128,
                        num_idxs_reg=gvalid[gci + gi],
                        elem_size=2 * F, elem_step=64,
                        single_packet=False,
                        queue_num=gq % 4)
                    gq += 1
                for B in range(b0, b1_):
                    o = boff[B] - c0
                    cap = caps[B]
                    gv = gchunk[:, o * 2 * F:(o + cap) * 2 * F].rearrange(
                        "p (c j f) -> p c j f", j=2, f=F)
                    mv = maskt[:, (c0 + o) * 2:(c0 + o + cap) * 2].rearrange(
                        "p (c j) -> p c j", j=2)
                    mvb = bass.AP(mv.tensor, mv.offset,
                                  list(mv.ap) + [[0, F]])
                    # in-place: select the right sub-row and fold dinv[dst]
                    nc.vector.tensor_tensor(out=gv, in0=gv, in1=mvb,
                                            op=OP.mult)
                    agg = workp.tile([128, F], f32, tag="agg")
                    nc.vector.tensor_reduce(
                        agg[:],
                        gchunk[:, o * 2 * F:(o + cap) * 2 * F].rearrange(
                            "p (s f) -> p f s", f=F),
                        axis=AX.X, op=OP.add)
                    pt = psump.tile([F, 128], f32, tag="pt")
                    nc.tensor.transpose(out=pt[:], in_=agg[:],
                                        identity=ident[:])
                    aggT = workp.tile([F, 128], f32, tag="aggT")
                    nc.scalar.copy(aggT[:], pt[:])
                    ph = psump.tile([128, H1], f32, tag="ph")
                    nc.tensor.matmul(out=ph[:], lhsT=aggT[:], rhs=w1[:],
                                     start=True, stop=False)
                    nc.tensor.matmul(out=ph[:], lhsT=ones_row[:], rhs=b1s[:],
                                     start=False, stop=True)
                    hd = workp.tile([128, H1], f32, tag="hd")
                    # relu(y * dinv) == dinv * relu(y) since dinv > 0
                    nc.scalar.activation(hd[:], ph[:], AF.Relu,
                                         scale=dinv[:, B:B + 1])
                    nc.tensor.matmul(out=psum_zT[:], lhsT=hd[:],
                                     rhs=call[:, B * G:(B + 1) * G],
                                     start=(ci == 0 and B == b0),
                                     stop=(ci == len(rep_chunks) - 1
                                           and B == b1_ - 1),
                                     skip_group_check=True)

            zT = constp.tile([H1, G], f32)
            nc.scalar.copy(zT[:], psum_zT[:])
            pP = psump.tile([G, H2], f32, tag="pP")
            nc.tensor.matmul(out=pP[:], lhsT=zT[:], rhs=w2[:],
                             start=True, stop=False)
            nc.tensor.matmul(out=pP[:], lhsT=ones_b2[:], rhs=b2s[:],
                             start=False, stop=True)
            pout = constp.tile([G, H2], f32)
            nc.scalar.activation(pout[:], pP[:], AF.Copy, scale=1.0 / GS)
            nc.sync.dma_start(out=out_t[:, :], in_=pout[:])

    nc.compile()
    return nc


_CACHE = {}


def kernel(**inputs):
    x = np.asarray(inputs["x"], dtype=np.float32)
    edge_index = np.asarray(inputs["edge_index"])
    W1 = np.asarray(inputs["W1"], dtype=np.float32)
    b1 = np.asarray(inputs["b1"], dtype=np.float32)
    W2 = np.asarray(inputs["W2"], dtype=np.float32)
    b2 = np.asarray(inputs["b2"], dtype=np.float32)
    assert x.shape == (N, F) and edge_index.shape == (2, E)

    cfg, per_core = _prep(x, edge_index, W1, b1, W2, b2)

    from concourse.bass_utils import run_bass_kernel_spmd

    if cfg not in _CACHE:
        _CACHE[cfg] = _build(cfg)
    nc = _CACHE[cfg]

    res = run_bass_kernel_spmd(nc, per_core, list(range(NCORES)))
    out = np.zeros((G, H2), np.float64)
    for r in res.results:
        out += r["p_out"].astype(np.float64)
    return out.astype(np.float32).reshape(1, G, H2)


if __name__ == "__main__":
    rng = np.random.default_rng(0)
    ins = dict(
        x=rng.standard_normal((N, F), dtype=np.float32),
        edge_index=rng.integers(0, N, (2, E)).astype(np.int32),
        W1=rng.standard_normal((F, H1), dtype=np.float32) * 0.25,
        b1=np.zeros(H1, np.float32),
        W2=rng.standard_normal((H1, H2), dtype=np.float32) * 0.125,
        b2=np.zeros(H2, np.float32),
    )
    out = kernel(**ins)
    print(out.shape, out.dtype, float(np.abs(out).mean()))
